# revision 1
# baseline (speedup 1.0000x reference)
"""Trainium2 Bass kernel for nn_AttnMatching.

Reference computes:
    emb = emb_table[1:L+1]                      # [L, D]
    attn = einsum('ld,ntd->nlt', emb, self_attn)
    out  = einsum('nlt,t->nl', attn, value_w[0])

Reassociated (identical math, fp32):
    ctx[n, d] = sum_t value_w[t] * self_attn[n, t, d]    # [N, D]  (tiny)
    out[n, l] = sum_d ctx[n, d] * emb[l, d]              # [N, L]

Memory-bound: dominant traffic is streaming the 25.6 MB embedding table.
Sharding: vocab axis L split across 8 cores (6250 cols each),
self_attn/value_w replicated, no communication. Host-side marshalling
puts each tensor in its DMA-friendly layout:
  - emb shard pre-transposed to [D=128, Lsh] (contraction dim on
    partitions; large per-partition descriptors per chunk).
  - self_attn re-laid-out d-major as attn_dT[d, n*T+t] with value_w
    broadcast to [D, T] prepended -> one [128, 1700] region with
    contiguous 6.8 KB per-partition rows.

Per-core program (default raw bacc implementation, hand-rolled sems;
a TileContext variant is kept behind K_IMPL=tile):
  - attn+w bursts first on the sync HWDGE ring (ring FIFO gives it a
    solo full-rate window); sync then streams half the emb chunks,
    gpsimd (SWDGE) streams the rest once attnw has landed.
  - ctxT[d, n] built on the DVE: one fused multiply + free-dim-reduce
    (scalar_tensor_tensor accum_out) per batch row, pipelined behind
    the attnw sub-DMAs.
  - PE: dependency-free bf16 warmup matmuls hold the HAM at 2.4 GHz,
    then fp32 mains: lhsT=ctxT [D,16] stationary, rhs = emb chunks
    [D,<=512] -> PSUM [16,<=512] -> DVE copy -> chunked store DMA on
    the scalar ring.
  - Epilogue: sem-only all-engine barrier + semaphore range clear so
    the NEFF is safe to re-execute.
"""

import os

import numpy as np

L = 50000
D = 128
T = 100
N = 16
NCORES = 8
LSH = L // NCORES          # 6250 columns per core

# knobs (env-overridable for A/B profiling)
DMA_CHUNK = int(os.environ.get("K_DMA_CHUNK", "1024"))  # emb load granularity
MM_CHUNK = 512             # matmul moving-operand / PSUM bank limit
MM_DT = os.environ.get("K_MM_DT", "float32")  # matmul input dtype mode
NUM_DEVICES = int(os.environ.get("K_NUM_DEVICES", str(NCORES)))
N_WARMUP = int(os.environ.get("K_N_WARMUP", "8"))  # PE HAM warmup matmuls
IMPL = os.environ.get("K_IMPL", "raw")  # "tile" | "raw"

_cache = {}


def _chunks(total, step):
    return [(c0, min(c0 + step, total)) for c0 in range(0, total, step)]


def _build():
    import concourse.bacc as bacc
    import concourse.mybir as mybir
    import concourse.tile as tile

    mm_dt = getattr(mybir.dt, MM_DT)

    nc = bacc.Bacc(
        "TRN2",
        target_bir_lowering=False,
        debug=False,
        enable_asserts=True,
        num_devices=NUM_DEVICES,
    )

    embT = nc.dram_tensor("embT", [D, LSH], mm_dt, kind="ExternalInput").ap()
    attnw = nc.dram_tensor(
        "attnw", [T, N * D + 1], mybir.dt.float32, kind="ExternalInput"
    ).ap()
    out = nc.dram_tensor("out", [N, LSH], mybir.dt.float32, kind="ExternalOutput").ap()

    from concourse.tile_rust import add_dep_helper

    dma_chunks = _chunks(LSH, DMA_CHUNK)
    n_sync = (len(dma_chunks) + 1) // 2

    with tile.TileContext(nc) as tc:
        with (
            tc.tile_pool(name="consts", bufs=1) as consts,
            tc.tile_pool(name="embp", bufs=len(dma_chunks)) as embp,
            tc.tile_pool(name="outp", bufs=3) as outp,
            tc.tile_pool(name="psc", bufs=1, space="PSUM") as psc,
            tc.tile_pool(name="pso", bufs=4, space="PSUM") as pso,
        ):
            # attn+w upload, layout [w | n0..n15 blocks], split into 4
            # sub-DMAs issued FIRST on the sync ring: ring FIFO gives them
            # a solo full-rate burst before the emb stream, and the ctx
            # matmuls pipeline behind the sub-DMAs via subtile deps.
            attnw_tile = consts.tile([T, N * D + 1], mybir.dt.float32)
            attnw_bounds = [0, 513, 1025, 1537, 2049]
            attnw_last = None
            for a0, a1 in zip(attnw_bounds[:-1], attnw_bounds[1:]):
                attnw_last = nc.sync.dma_start(
                    attnw_tile[:, a0:a1], attnw[:, a0:a1]
                )

            # emb chunks: first half behind attnw on the sync ring (FIFO);
            # rest on the gpsimd ring, dep-delayed behind the attnw burst
            # so round-robin doesn't starve it.
            emb_tiles = []
            for ci, (c0, c1) in enumerate(dma_chunks):
                et = embp.tile(
                    [D, c1 - c0], mm_dt, tag="emb", name=f"emb_{c0}"
                )
                eng = nc.sync if ci < n_sync else nc.gpsimd
                dma = eng.dma_start(et[:, :], embT[:, c0:c1])
                if ci == n_sync:
                    add_dep_helper(
                        attnw_last.ins, dma.ins, sync=True,
                        reason="gpsimd emb stream waits for attnw burst",
                    )
                emb_tiles.append(et)

            # PE HAM warmup: dependency-free bf16 matmuls on a zeroed
            # scratch keep the PE at 2.4 GHz until real matmuls arrive.
            if N_WARMUP:
                wscr = consts.tile([D, D + MM_CHUNK], mybir.dt.bfloat16)
                nc.vector.memset(wscr[:, :], 0.0)
                ps_w = psc.tile(
                    [D, MM_CHUNK], mybir.dt.float32, tag="ps_warm", name="ps_warm"
                )
                for wi in range(N_WARMUP):
                    nc.tensor.matmul(
                        ps_w[:, :],
                        lhsT=wscr[:, :D],
                        rhs=wscr[:, D:],
                        start=True,
                        stop=True,
                    )

            # ctxT[d, n] = sum_t self_attn[n, t, d] * w[t]
            ps_ctx = psc.tile([D, N], mybir.dt.float32)
            for n in range(N):
                nc.tensor.matmul(
                    ps_ctx[:, n : n + 1],
                    lhsT=attnw_tile[:, 1 + n * D : 1 + (n + 1) * D],
                    rhs=attnw_tile[:, 0:1],
                    start=True,
                    stop=True,
                )
            ctxT = consts.tile([D, N], mm_dt)
            nc.vector.tensor_copy(ctxT[:, :], ps_ctx[:, :])
            ctxT_mm = ctxT[:, :]

            # out[n, c0:c1] = ctxT.T @ embT[:, c0:c1]
            for ci, (c0, c1) in enumerate(dma_chunks):
                ot = outp.tile([N, c1 - c0], mybir.dt.float32, tag="out", name=f"out_{c0}")
                for s0, s1 in _chunks(c1 - c0, MM_CHUNK):
                    ps = pso.tile(
                        [N, s1 - s0], mybir.dt.float32, tag="pso", name=f"ps_{c0}_{s0}"
                    )
                    nc.tensor.matmul(
                        ps[:, :],
                        lhsT=ctxT_mm,
                        rhs=emb_tiles[ci][:, s0:s1],
                        start=True,
                        stop=True,
                    )
                    nc.vector.tensor_copy(ot[:, s0:s1], ps[:, :])
                nc.scalar.dma_start(out[:, c0:c1], ot[:, :])

    nc.compile()
    return nc


def _build_raw():
    """Raw bacc (no TileContext): hand-rolled semaphores, same schedule as
    the Tile build but with a minimal prologue/epilogue."""
    import concourse.bacc as bacc
    import concourse.mybir as mybir

    f32 = mybir.dt.float32
    bf16 = mybir.dt.bfloat16

    nc = bacc.Bacc(
        "TRN2",
        target_bir_lowering=False,
        debug=False,
        enable_asserts=True,
        num_devices=NUM_DEVICES,
    )

    embT = nc.dram_tensor("embT", [D, LSH], f32, kind="ExternalInput").ap()
    # [D, T + N*T]: cols 0..T-1 = value_w broadcast over partitions,
    # cols T.. = self_attn in d-major layout attn_dT[d, n*T+t].
    AW = T + N * T
    attnw = nc.dram_tensor("attnw", [D, AW], f32, kind="ExternalInput").ap()
    out = nc.dram_tensor("out", [N, LSH], f32, kind="ExternalOutput").ap()

    # First emb chunk small (one matmul's worth): it queues behind the
    # attnw burst on the sync ring and gates the first main matmul, so
    # keep its transfer short.
    dma_chunks = [(0, MM_CHUNK)] + [
        (c0 + MM_CHUNK, c1 + MM_CHUNK) for c0, c1 in _chunks(LSH - MM_CHUNK, DMA_CHUNK)
    ]
    n_chunks = len(dma_chunks)
    n_sync = (n_chunks + 1) // 2
    # two attnw sub-DMAs: 3.4 KB per-partition descriptors stream faster
    # than a 4-way split's 1.7 KB ones, and ctx ops still pipeline.
    attnw_bounds = [0, T + 8 * T, AW]
    n_sub = len(attnw_bounds) - 1
    CTX_PER_SUB = N // n_sub
    # gpsimd emb stream starts once this attnw sub-DMA has landed
    GP_DELAY_SUB = int(os.environ.get("K_GP_DELAY_SUB", str(n_sub - 1)))
    # global matmul list: (chunk_idx, abs_s0, abs_s1)
    mm_list = []
    for ci, (c0, c1) in enumerate(dma_chunks):
        for s0, s1 in _chunks(c1 - c0, MM_CHUNK):
            mm_list.append((ci, c0 + s0, c0 + s1))
    NPS = 4

    attnw_sb = nc.alloc_sbuf_tensor("attnw_sb", [D, AW], f32).ap()
    emb_sb = [
        nc.alloc_sbuf_tensor(f"emb_sb{ci}", [D, c1 - c0], f32).ap()
        for ci, (c0, c1) in enumerate(dma_chunks)
    ]
    out_sb = nc.alloc_sbuf_tensor("out_sb", [N, LSH], f32).ap()
    wscr = nc.alloc_sbuf_tensor("wscr", [D, D + MM_CHUNK], bf16).ap()
    ctxT = nc.alloc_sbuf_tensor("ctxT", [D, N], f32).ap()
    ctx_scr = nc.alloc_sbuf_tensor("ctx_scr", [D, N * T], f32).ap()
    ps_warm = nc.alloc_psum_tensor("ps_warm", [D, MM_CHUNK], f32).ap()
    ps_main = [
        nc.alloc_psum_tensor(f"ps_main{j}", [N, MM_CHUNK], f32).ap()
        for j in range(NPS)
    ]

    lda = [nc.alloc_semaphore(f"lda{g}") for g in range(n_sub)]
    lde = [nc.alloc_semaphore(f"lde{ci}") for ci in range(n_chunks)]
    z = nc.alloc_semaphore("z")
    cc = nc.alloc_semaphore("cc")
    mm = nc.alloc_semaphore("mm")
    cp = nc.alloc_semaphore("cp")
    st = nc.alloc_semaphore("st")
    all_sems = lda + lde + [z, cc, mm, cp, st]

    # SP load issues + DVE warmup-scratch memset go in the entry block,
    # BEFORE nc.Block(): they run right after the boot barrier instead of
    # paying the block branch + IRAM fetch first (same pattern as
    # bass_test_utils.run_sbuf_kernel's pre-block loads).
    for g, (a0, a1) in enumerate(zip(attnw_bounds[:-1], attnw_bounds[1:])):
        nc.sync.dma_start(attnw_sb[:, a0:a1], attnw[:, a0:a1]).then_inc(lda[g], 16)
    for ci in range(n_sync):
        c0, c1 = dma_chunks[ci]
        nc.sync.dma_start(emb_sb[ci][:, :], embT[:, c0:c1]).then_inc(lde[ci], 16)
    nc.vector.memset(wscr[:, :], 0.0).then_inc(z, 1)

    with nc.Block() as block:

        @block.gpsimd
        def _(gp):
            # don't compete with the attnw burst
            gp.wait_ge(lda[GP_DELAY_SUB], 16)
            for ci in range(n_sync, n_chunks):
                c0, c1 = dma_chunks[ci]
                gp.dma_start(emb_sb[ci][:, :], embT[:, c0:c1]).then_inc(
                    lde[ci], 16
                )

        @block.vector
        def _(v):
            # ctxT[:, n] = sum_t attn_dT[:, n*T+t] * w[t] — one fused
            # multiply+freedim-reduce per n on the DVE.
            for nidx in range(N):
                if nidx % CTX_PER_SUB == 0:
                    v.wait_ge(lda[nidx // CTX_PER_SUB], 16)
                inst = nc.vector.scalar_tensor_tensor(
                    ctx_scr[:, nidx * T : (nidx + 1) * T],
                    attnw_sb[:, T + nidx * T : T + (nidx + 1) * T],
                    1.0,
                    attnw_sb[:, 0:T],
                    op0=mybir.AluOpType.bypass,
                    op1=mybir.AluOpType.mult,
                    accum_out=ctxT[:, nidx : nidx + 1],
                )
            inst.then_inc(cc, 1)
            for s, (ci, s0, s1) in enumerate(mm_list):
                v.wait_ge(mm, s + 1)
                nc.vector.tensor_copy(
                    out_sb[:, s0:s1], ps_main[s % NPS][:, : s1 - s0]
                ).then_inc(cp, 1)

        @block.tensor
        def _(t):
            t.wait_ge(z, 1)
            for _wi in range(N_WARMUP):
                nc.tensor.matmul(
                    ps_warm[:, :],
                    lhsT=wscr[:, :D],
                    rhs=wscr[:, D:],
                    start=True,
                    stop=True,
                )
            t.wait_ge(cc, 1)
            prev_ci = -1
            for s, (ci, s0, s1) in enumerate(mm_list):
                if ci != prev_ci:
                    t.wait_ge(lde[ci], 16)
                    prev_ci = ci
                if s >= NPS:
                    t.wait_ge(cp, s - NPS + 1)
                c0 = dma_chunks[ci][0]
                nc.tensor.matmul(
                    ps_main[s % NPS][:, : s1 - s0],
                    lhsT=ctxT[:, :],
                    rhs=emb_sb[ci][:, s0 - c0 : s1 - c0],
                    start=True,
                    stop=True,
                ).then_inc(mm, 1)

        @block.scalar
        def _(sc):
            copies_done = 0
            for ci, (c0, c1) in enumerate(dma_chunks):
                copies_done += len(_chunks(c1 - c0, MM_CHUNK))
                sc.wait_ge(cp, copies_done)
                sc.dma_start(out[:, c0:c1], out_sb[:, c0:c1]).then_inc(st, 16)
            # no completion wait here: the epilogue's clear_and_free
            # dma_reset drains the st-associated store queue on gpsimd
            # before the NEFF can complete, guaranteeing the writes land.

    # epilogue: quiesce engines, zero sems for re-execution safety
    nc.all_engine_barrier(sem_only=True)
    nc.clear_and_free_semaphores(all_sems)

    nc.compile()
    return nc


def _get_nc():
    if "nc" not in _cache:
        _cache["nc"] = _build_raw() if IMPL == "raw" else _build()
    return _cache["nc"]


def _make_in_maps(self_attn, emb_table, value_w):
    self_attn = np.asarray(self_attn, dtype=np.float32)
    value_w = np.asarray(value_w, dtype=np.float32)
    if IMPL == "raw":
        # [D, T + N*T]: value_w broadcast, then d-major self_attn
        attnw = np.empty((D, T + N * T), dtype=np.float32)
        attnw[:, :T] = value_w[0][None, :]
        attnw[:, T:] = self_attn.transpose(2, 0, 1).reshape(D, N * T)
    else:
        # [T, 1 + N*D]: value_w first, then transposed self_attn blocks
        attnw = np.empty((T, N * D + 1), dtype=np.float32)
        attnw[:, 0] = value_w[0]
        attnw[:, 1:] = self_attn.transpose(1, 0, 2).reshape(T, N * D)
    embT = np.asarray(emb_table, dtype=np.float32)[1 : L + 1].T  # [D, L]
    return [
        {
            "embT": np.ascontiguousarray(embT[:, k * LSH : (k + 1) * LSH]),
            "attnw": attnw,
        }
        for k in range(NCORES)
    ]


def run(self_attn, emb_table, value_w, trace=False):
    from concourse.bass_utils import run_bass_kernel_spmd

    nc = _get_nc()
    in_maps = _make_in_maps(self_attn, emb_table, value_w)
    res = run_bass_kernel_spmd(nc, in_maps, list(range(NCORES)), trace=trace)
    full = np.concatenate(
        [res.results[k]["out"] for k in range(NCORES)], axis=1
    ).astype(np.float32)
    return full, res


def kernel(self_attn, mat2, traj, emb_table, value_w):
    full, _ = run(self_attn, emb_table, value_w, trace=False)
    return full



# revision 12
# speedup vs baseline: 1.3584x; 1.3584x over previous
"""Trainium2 Bass kernel for nn_AttnMatching.

Reference computes:
    emb = emb_table[1:L+1]                      # [L, D]
    attn = einsum('ld,ntd->nlt', emb, self_attn)
    out  = einsum('nlt,t->nl', attn, value_w[0])

Reassociated (identical math):
    ctx[n, d] = sum_t value_w[t] * self_attn[n, t, d]    # [N, D]  (tiny)
    out[n, l] = sum_d ctx[n, d] * emb[l, d]              # [N, L]

Memory-bound: dominant traffic is streaming the embedding table.
Sharding: vocab axis L split across 8 cores (6250 cols each),
self_attn/value_w replicated, no communication.

All device traffic is bf16 (host-cast): emb 1.6 MB/core, attnw 0.44 MB,
out 0.2 MB. Matmuls run at bf16 rate (fp32 is 4 cycles/row + LOW_HIGH
double-pass). rel-err from bf16 ~3e-3, gate is 2e-2.

Per-core schedule (raw bacc, hand-rolled sems):
  - 3 DMA queues stream from the entry block: sync(HWDGE) carries the
    attnw burst (2 subs) then the last emb chunk; scalar(HWDGE) the
    first two emb chunks; gpsimd(SWDGE) the middle three.
  - ctx: per 8-n half, ONE fused multiply (scalar_tensor_tensor with a
    zero-stride broadcast AP repeating the w block) + ONE segmented
    tensor_reduce (axis=X over [128,8,100]) on DVE; ACT casts ctx_f32
    -> bf16 ctxT cross-engine (sem-gated: accumulator/pipeline drain).
  - PE: dependency-free bf16 warmups hold the clock ramp, then mains.
    Two schemes (K_SCHEME):
      wide: lhsT=ctxT [D,16] stationary, rhs=emb [D,512] -> PSUM
            [16,512] x13 over 6 banks; PSUM->SBUF copies round-robin
            DVE/ACT; out_sb repartitioned [64, 2048] (copy s writes
            partition base 16*(s%4)) so stores engage 8 SDMA engines.
      tp:   lhsT=emb tile [D,128] stationary, rhs=ctxT [D,16] moving ->
            PSUM [128,16] x49 packed into 2 banks; 2 full-width DVE
            copies; 2 full-rate [128,*] stores (host un-permutes).
  - Epilogue: sem-only all-engine barrier + semaphore clear so the NEFF
    is safe to re-execute.
"""

import os

import numpy as np

L = 50000
D = 128
T = 100
N = 16
NCORES = 8
LSH = L // NCORES          # 6250 columns per core

MM = 512                   # PSUM bank limit: fp32 out cols per matmul
SCHEME = os.environ.get("K_SCHEME", "wide")  # "wide" | "tp"
N_WARMUP = int(os.environ.get("K_N_WARMUP", "10"))
NUM_DEVICES = int(os.environ.get("K_NUM_DEVICES", str(NCORES)))
# wide-scheme out_sb layout: "shift" = [64, 2048] repartitioned (8-engine
# stores), "flat" = [16, LSH] (4-engine stores, no partition-shift copies)
OSB = os.environ.get("K_OSB", "shift")

AW = T + N * T             # attnw cols: [w bcast | sa d-major]
NTILE = (LSH + 127) // 128          # 49 transposed tiles
LPAD = NTILE * 128                  # 6272: tp-scheme padded cols
NCOLS = {"wide": LSH, "tp": LPAD}

_cache = {}


def _chunks(total, step):
    return [(c0, min(c0 + step, total)) for c0 in range(0, total, step)]


def _view3(ap2d, ncols_inner, nrep, bcast=False):
    """[128, nrep*ncols_inner] slice -> [128, nrep, ncols_inner] view.
    bcast repeats the first ncols_inner cols nrep times (stride 0)."""
    from concourse.bass import AP

    pstep = ap2d.ap[0][0]
    step_rep = 0 if bcast else ncols_inner
    return AP(
        ap2d.tensor,
        ap2d.offset,
        [[pstep, 128], [step_rep, nrep], [1, ncols_inner]],
    )


def _build():
    import concourse.bacc as bacc
    import concourse.mybir as mybir

    f32 = mybir.dt.float32
    bf16 = mybir.dt.bfloat16

    nc = bacc.Bacc(
        "TRN2",
        target_bir_lowering=False,
        debug=False,
        enable_asserts=True,
        num_devices=NUM_DEVICES,
    )

    ncols = NCOLS[SCHEME]
    embT = nc.dram_tensor("embT", [D, ncols], bf16, kind="ExternalInput").ap()
    attnw = nc.dram_tensor("attnw", [D, AW], bf16, kind="ExternalInput").ap()
    if SCHEME == "wide":
        out_shape = [128, 2048] if OSB == "shift" else [N, LSH]
    else:
        out_shape = [D, NTILE * N]
    out = nc.dram_tensor("out", out_shape, bf16, kind="ExternalOutput").ap()

    # emb chunks across the 3 DMA queues (1024-col granularity)
    bounds = [0, 1024, 2048, 3072, 4096, 5120, ncols]
    dma_chunks = list(zip(bounds[:-1], bounds[1:]))
    ring = {0: "scalar", 1: "scalar", 2: "gpsimd", 3: "gpsimd", 4: "gpsimd", 5: "sync"}
    n_chunks = len(dma_chunks)

    # attnw sub-DMAs: sub0 = w + n0..7, sub1 = n8..15
    asub = [0, T + 8 * T, AW]

    # wide-scheme matmul list: (chunk_idx, abs_s0, abs_s1), 512-col units
    mm_list = []
    for ci, (c0, c1) in enumerate(dma_chunks):
        for s0, s1 in _chunks(c1 - c0, MM):
            mm_list.append((ci, c0 + s0, c0 + s1))
    NPS = 6
    CPENG = ["vector", "scalar"]  # copy engine per wide mm-unit (no PSUM on gpsimd)
    ne = len(CPENG)

    attnw_sb = nc.alloc_sbuf_tensor("attnw_sb", [D, AW], bf16).ap()
    emb_sb = [
        nc.alloc_sbuf_tensor(f"emb_sb{ci}", [D, c1 - c0], bf16).ap()
        for ci, (c0, c1) in enumerate(dma_chunks)
    ]
    wscr = nc.alloc_sbuf_tensor("wscr", [D, D + MM], bf16).ap()
    ctxT = nc.alloc_sbuf_tensor("ctxT", [D, N], bf16).ap()
    ctx_f32 = nc.alloc_sbuf_tensor("ctx_f32", [D, N], f32).ap()
    prod = nc.alloc_sbuf_tensor("prod", [D, 8 * T], bf16).ap()
    if SCHEME == "wide":
        out_sb = nc.alloc_sbuf_tensor("out_sb", out_shape, bf16).ap()
    else:
        out_sb = nc.alloc_sbuf_tensor("out_sb", [D, NTILE * N], bf16).ap()

    ps_warm = nc.alloc_psum_tensor("ps_warm", [D, MM], f32).ap()
    if SCHEME == "wide":
        ps_main = [
            nc.alloc_psum_tensor(f"ps_main{j}", [N, MM], f32).ap() for j in range(NPS)
        ]
    else:
        ps_tp = [
            nc.alloc_psum_tensor("ps_tp0", [D, MM], f32).ap(),
            nc.alloc_psum_tensor("ps_tp1", [D, (NTILE - 32) * N], f32).ap(),
        ]

    lda = [nc.alloc_semaphore(f"lda{g}") for g in range(2)]
    lde = [nc.alloc_semaphore(f"lde{ci}") for ci in range(n_chunks)]
    z = nc.alloc_semaphore("z")
    cxr = nc.alloc_semaphore("cxr")
    cxv = nc.alloc_semaphore("cxv")
    mm_sem = nc.alloc_semaphore("mm")
    cp = {k: nc.alloc_semaphore(f"cp_{k}") for k in CPENG}
    st = nc.alloc_semaphore("st")
    all_sems = lda + lde + [z, cxr, cxv, mm_sem] + list(cp.values()) + [st]

    # ---- entry block: all unconditional DMA issues + warmup memset ----
    for g in range(2):
        nc.sync.dma_start(
            attnw_sb[:, asub[g] : asub[g + 1]], attnw[:, asub[g] : asub[g + 1]]
        ).then_inc(lda[g], 16)
    for ci, (c0, c1) in enumerate(dma_chunks):
        eng = getattr(nc, ring[ci])
        eng.dma_start(emb_sb[ci][:, :], embT[:, c0:c1]).then_inc(lde[ci], 16)
    nc.vector.memset(wscr[:, :], 0.0).then_inc(z, 1)

    def ctx_half(h):
        """prod = attnw[n-half h] * w (one STT), ctx_f32 half = segmented sum."""
        in0 = _view3(attnw_sb[:, T + h * 8 * T : T + (h + 1) * 8 * T], T, 8)
        w_b = _view3(attnw_sb[:, 0:T], T, 8, bcast=True)
        pv = _view3(prod[:, :], T, 8)
        nc.vector.scalar_tensor_tensor(
            pv, in0, 1.0, w_b,
            op0=mybir.AluOpType.bypass,
            op1=mybir.AluOpType.mult,
        )
        return nc.vector.tensor_reduce(
            ctx_f32[:, h * 8 : (h + 1) * 8], pv,
            axis=mybir.AxisListType.X, op=mybir.AluOpType.add,
        )

    # wide+shift: copy for mm-unit s lands at partition base 32*(s%4) (engine
    # writes must start on a quadrant boundary), col block 512*(s//4); stores
    # then read all partition quadrants (16 SDMA engines, half-garbage rows)
    def osb_dst(s, width):
        if OSB == "shift":
            a, b = s % 4, s // 4
            return out_sb[32 * a : 32 * a + 16, MM * b : MM * b + width]
        ci, s0, s1 = mm_list[s]
        return out_sb[:, s0 : s0 + width]

    with nc.Block() as block:

        @block.vector
        def _(v):
            v.wait_ge(lda[0], 16)
            ctx_half(0)
            v.wait_ge(lda[1], 16)
            ctx_half(1).then_inc(cxr, 1)
            if SCHEME == "wide":
                for s, (ci, s0, s1) in enumerate(mm_list):
                    if CPENG[s % ne] != "vector":
                        continue
                    v.wait_ge(mm_sem, s + 1)
                    nc.vector.tensor_copy(
                        osb_dst(s, s1 - s0), ps_main[s % NPS][:, : s1 - s0]
                    ).then_inc(cp["vector"], 1)
            else:
                v.wait_ge(mm_sem, 32)
                nc.vector.tensor_copy(out_sb[:, : 32 * N], ps_tp[0][:, :]).then_inc(
                    cp["vector"], 1
                )
                v.wait_ge(mm_sem, NTILE)
                nc.vector.tensor_copy(out_sb[:, 32 * N :], ps_tp[1][:, :]).then_inc(
                    cp["vector"], 1
                )

        @block.tensor
        def _(t):
            t.wait_ge(z, 1)
            for _wi in range(N_WARMUP):
                nc.tensor.matmul(
                    ps_warm[:, :],
                    lhsT=wscr[:, :D],
                    rhs=wscr[:, D:],
                    start=True,
                    stop=True,
                )
            t.wait_ge(cxv, 1)
            if SCHEME == "wide":
                prev_ci = -1
                for s, (ci, s0, s1) in enumerate(mm_list):
                    if ci != prev_ci:
                        t.wait_ge(lde[ci], 16)
                        prev_ci = ci
                    if s >= NPS:
                        q = s - NPS
                        t.wait_ge(cp[CPENG[q % ne]], q // ne + 1)
                    c0 = dma_chunks[ci][0]
                    nc.tensor.matmul(
                        ps_main[s % NPS][:, : s1 - s0],
                        lhsT=ctxT[:, :],
                        rhs=emb_sb[ci][:, s0 - c0 : s1 - c0],
                        start=True,
                        stop=True,
                    ).then_inc(mm_sem, 1)
            else:
                prev_ci = -1
                for ti in range(NTILE):
                    ci = min(ti // 8, n_chunks - 1)
                    if ci != prev_ci:
                        t.wait_ge(lde[ci], 16)
                        prev_ci = ci
                    c0 = dma_chunks[ci][0]
                    bank, off = (0, ti) if ti < 32 else (1, ti - 32)
                    nc.tensor.matmul(
                        ps_tp[bank][:, off * N : (off + 1) * N],
                        lhsT=emb_sb[ci][:, ti * 128 - c0 : ti * 128 - c0 + 128],
                        rhs=ctxT[:, :],
                        start=True,
                        stop=True,
                    ).then_inc(mm_sem, 1)

        @block.scalar
        def _(sc):
            # ctx cast runs cross-engine: guarantees DVE reduce has drained
            sc.wait_ge(cxr, 1)
            nc.scalar.copy(ctxT[:, :], ctx_f32[:, :]).then_inc(cxv, 1)
            if SCHEME == "wide":
                acts = [s for s in range(len(mm_list)) if CPENG[s % ne] == "scalar"]
                if OSB == "shift":
                    # store1: mm-units 0-7 (col blocks 0-1); store2: 8-12
                    store_plan = [(0, 8, 0, 1024), (8, 13, 1024, 2048)]
                else:
                    store_plan = [(0, 3, 0, 1536), (3, 6, 1536, 3072),
                                  (6, 9, 3072, 4608), (9, 13, 4608, LSH)]
                done = {k: 0 for k in CPENG}
                ai = 0
                for q0, q1, b0, b1 in store_plan:
                    while ai < len(acts) and acts[ai] < q1:
                        s = acts[ai]
                        ci, s0, s1 = mm_list[s]
                        sc.wait_ge(mm_sem, s + 1)
                        nc.scalar.copy(
                            osb_dst(s, s1 - s0), ps_main[s % NPS][:, : s1 - s0]
                        ).then_inc(cp["scalar"], 1)
                        done["scalar"] += 1
                        ai += 1
                    for s in range(q0, q1):
                        k = CPENG[s % ne]
                        need = s // ne + 1
                        if k != "scalar" and need > done[k]:
                            sc.wait_ge(cp[k], need)
                            done[k] = need
                    if OSB == "shift":
                        nc.scalar.dma_start(
                            out[:, b0:b1], out_sb[:, b0:b1]
                        ).then_inc(st, 16)
                    else:
                        nc.scalar.dma_start(
                            out[:, b0:b1], out_sb[:, b0:b1]
                        ).then_inc(st, 16)
            else:
                sc.wait_ge(cp["vector"], 1)
                nc.scalar.dma_start(out[:, : 32 * N], out_sb[:, : 32 * N]).then_inc(
                    st, 16
                )
                sc.wait_ge(cp["vector"], 2)
                nc.scalar.dma_start(out[:, 32 * N :], out_sb[:, 32 * N :]).then_inc(
                    st, 16
                )
            # no completion wait: epilogue dma_reset drains the store queue

    nc.all_engine_barrier(sem_only=True)
    nc.clear_and_free_semaphores(all_sems)

    nc.compile()
    return nc


def _get_nc():
    if "nc" not in _cache:
        _cache["nc"] = _build()
    return _cache["nc"]


def _make_in_maps(self_attn, emb_table, value_w):
    import ml_dtypes

    bf = ml_dtypes.bfloat16
    self_attn = np.asarray(self_attn, dtype=np.float32)
    value_w = np.asarray(value_w, dtype=np.float32)
    # [D, T + N*T]: value_w broadcast, then d-major self_attn
    attnw = np.empty((D, AW), dtype=bf)
    attnw[:, :T] = value_w[0][None, :].astype(bf)
    attnw[:, T:] = self_attn.transpose(2, 0, 1).reshape(D, N * T).astype(bf)
    embT = np.asarray(emb_table, dtype=np.float32)[1 : L + 1].astype(bf).T  # [D, L]
    ncols = NCOLS[SCHEME]
    in_maps = []
    for k in range(NCORES):
        shard = np.zeros((D, ncols), dtype=bf)
        shard[:, :LSH] = embT[:, k * LSH : (k + 1) * LSH]
        in_maps.append({"embT": shard, "attnw": attnw})
    return in_maps


def _unshard(o):
    o = np.asarray(o)
    if SCHEME == "wide":
        if OSB == "shift":
            # dram[32a+n, 512b+j] = out[n, 512*(4b+a)+j]
            full = np.empty((N, LSH), dtype=np.float32)
            for s in range(13):
                a, b = s % 4, s // 4
                w = min(MM, LSH - s * MM)
                full[:, s * MM : s * MM + w] = o[
                    32 * a : 32 * a + 16, MM * b : MM * b + w
                ].astype(np.float32)
            return full
        return o.astype(np.float32)
    # tp: [128, 49*16] -> [49,128,16] l-major -> [LSH, N] -> [N, LSH]
    return (
        o.reshape(D, NTILE, N)
        .transpose(1, 0, 2)
        .reshape(LPAD, N)[:LSH]
        .T.astype(np.float32)
    )


def run(self_attn, emb_table, value_w, trace=False):
    from concourse.bass_utils import run_bass_kernel_spmd

    nc = _get_nc()
    in_maps = _make_in_maps(self_attn, emb_table, value_w)
    res = run_bass_kernel_spmd(nc, in_maps, list(range(NCORES)), trace=trace)
    full = np.ascontiguousarray(
        np.concatenate([_unshard(res.results[k]["out"]) for k in range(NCORES)], axis=1),
        dtype=np.float32,
    )
    return full, res


def kernel(self_attn, mat2, traj, emb_table, value_w):
    full, _ = run(self_attn, emb_table, value_w, trace=False)
    return full
